# revision 13
# baseline (speedup 1.0000x reference)
"""CAAN attention-scorer kernel for 8 Trainium2 NeuronCores.

scores = relu(softmax(QK^T/sqrt(D)) @ V @ W1 + b1) @ W2 + b2
with Q/K/V = x @ W{q,k,v} + b{q,k,v};  N=8192, IN_DIM=1024, D=512.

Row-parallel attention with SHARDED K/V projections + AllGather:

  - Core c computes kT/v only for its own j-block (1/8 of N) and
    AllGathers the fp8 results (1 MB in -> 8 MB out).  This removes the
    448 replicated DoubleRow matmul slots (~100 us) the old kernel spent
    projecting all of K/V on every core.
  - The collective's latency (~40 us barrier/skew + ~40 us data) is
    hidden behind a small replicated PREFIX: every core also projects
    blocks 0..M-1 locally (consumed first in phase 2, both i-halves,
    before any gathered data is touched).
  - Phase 2 runs in 4 superchunks (ih0/ih1 x prefix/rest) accumulating
    partial ctx/den sums into SBUF f32 so both i-halves' prefix work can
    run before the AllGather lands (PSUM can't hold both halves at once).
  - Softmax normalization is deferred THROUGH the MLP's first matmul:
    g = W1^T ctxU (unnormalized), h = relu(g * recip + b1).  The
    reciprocal + gpsimd partition-broadcast run off the PE critical path.

Everything runs in a transposed layout so softmax denominators come from a
constant-column PE matmul:

  kT[d, j]   = (8 Wk)^T x^T (+8 bk)   d on partitions   (fp8, x8 scale)
  qT[d, i]   = (8 Wq)^T xq^T (+8 bq)                    (fp8, x8 scale)
  v[j, d]    = x (8 Wv) (+8 bv)       j on partitions   (fp8)
  S^T[j, i]  = kT-chunk^T qT          (= 64 * true scores)
  E          = exp(S^T / (64 sqrt(D)))     PSUM -> SBUF fp8
  ctxU^T    += v-chunk^T E            (= 8 * true ctxU)
  den[1,i]  += (8.0)^T E              (= 8 * true denom -> cancels the x8)
  g^T[m, i]  = W1-chunk^T ctxU^T      (bf16)
  h^T        = relu(g^T * (1/den) + b1)
  out[1, i]  = W2^T h^T + b2
"""

import numpy as np
import ml_dtypes

import concourse.tile as tile
from concourse import bacc, mybir
from concourse.bass_utils import run_bass_kernel_spmd

N, IN_DIM, D = 8192, 1024, 512
NCORES = 8
NB = N // NCORES            # 1024 rows per core
P = 128
KC = IN_DIM // P            # 8  k-chunks of the input dim
KP = KC // 2                # 4  DoubleRow k-pairs
DC = D // P                 # 4  d-chunks
DP = DC // 2                # 2  DoubleRow d-pairs
MC = (D // 2) // P          # 2  m-chunks of the hidden dim
JT = 512                    # j-tile width in phase 1
NJC = N // P                # 64 j-chunks
NJP = NJC // 2              # 32 j-pairs in phase 2
IH = 512                    # i-half width in phase 2
NIH = NB // IH              # 2
MPRE = 2                    # replicated prefix blocks (hide the AllGather)
WS = 8.0                    # fp8 weight pre-scale (exact power of two)
SCALE = 1.0 / float(np.sqrt(np.float32(D))) / (WS * WS)
BCOLS = DC * NB + 8 * D     # 8192 bounce cols: [kT share | v share]

FP8 = mybir.dt.float8e4
BF16 = mybir.dt.bfloat16
F32 = mybir.dt.float32
DR = mybir.MatmulPerfMode.DoubleRow

_CACHE = {}


def _build():
    nc = bacc.Bacc(None, target_bir_lowering=False, debug=False,
                   num_devices=NCORES)

    xq = nc.declare_dram_parameter("xq", [P, KC, NB], FP8, isOutput=False)
    xp = nc.declare_dram_parameter("xp", [P, KC, MPRE * NB], FP8, isOutput=False)
    wq = nc.declare_dram_parameter("wq", [P, KC, D], FP8, isOutput=False)
    wk = nc.declare_dram_parameter("wk", [P, KC, D], FP8, isOutput=False)
    wv = nc.declare_dram_parameter("wv", [P, KC, D], FP8, isOutput=False)
    w1 = nc.declare_dram_parameter("w1", [P, DC, MC, P], BF16, isOutput=False)
    w2 = nc.declare_dram_parameter("w2", [P, MC], BF16, isOutput=False)
    bq = nc.declare_dram_parameter("bq", [P, DC], F32, isOutput=False)   # x8
    bk = nc.declare_dram_parameter("bk", [P, DC], F32, isOutput=False)   # x8
    bv = nc.declare_dram_parameter("bv", [P, D], F32, isOutput=False)    # x8
    b1 = nc.declare_dram_parameter("b1", [P, MC], F32, isOutput=False)
    b2 = nc.declare_dram_parameter("b2", [1, 1], F32, isOutput=False)
    out = nc.declare_dram_parameter("out", [1, NB], F32, isOutput=True)

    with tile.TileContext(nc) as tc:
        with (
            tc.tile_pool(name="singles", bufs=1) as singles,
            tc.tile_pool(name="dram", bufs=1, space="DRAM") as dram,
        ):
            # ---- constants / weights into SBUF ----
            wq_sb = singles.tile([P, KC, D], FP8)
            wk_sb = singles.tile([P, KC, D], FP8)
            wv_sb = singles.tile([P, KC, D], FP8)
            w1_sb = singles.tile([P, DC, MC, P], BF16)
            w2_sb = singles.tile([P, MC], BF16)
            bq_sb = singles.tile([P, DC], F32)
            bk_sb = singles.tile([P, DC], F32)
            bv_sb = singles.tile([P, D], F32)
            b1_sb = singles.tile([P, MC], F32)
            b2_sb = singles.tile([1, 1], F32)
            cs_w = singles.tile([P, 2, 32], FP8)     # colsum weights = 8.0
            # (dual-fp8 ldweights needs >=32 active columns; rows identical)

            # critical-path DMAs first, spread across idle engine queues so
            # the first share matmul can start ~3us after the preamble
            xq_t0 = singles.tile([P, KC, JT], FP8)
            xq_t1 = singles.tile([P, KC, JT], FP8)
            nc.sync.dma_start(wk_sb[:, 0:2], wk[:, 0:2])
            nc.scalar.dma_start(xq_t0[:, 0:2], xq[:, 0:2, 0:JT])
            for kp in range(1, KP):
                s = slice(2 * kp, 2 * kp + 2)
                nc.scalar.dma_start(wk_sb[:, s], wk[:, s])
                nc.sync.dma_start(xq_t0[:, s], xq[:, s, 0:JT])
            nc.scalar.dma_start(bk_sb[:], bk[:])
            nc.scalar.dma_start(bv_sb[:], bv[:])
            for kp in range(KP):
                s = slice(2 * kp, 2 * kp + 2)
                eng = nc.sync if kp % 2 == 0 else nc.scalar
                eng.dma_start(xq_t1[:, s], xq[:, s, JT:2 * JT])
            nc.gpsimd.dma_start(out=wv_sb[:], in_=wv[:])
            nc.gpsimd.dma_start(out=wq_sb[:], in_=wq[:])
            nc.scalar.dma_start(bq_sb[:], bq[:])
            nc.scalar.dma_start(w1_sb[:], w1[:])
            nc.scalar.dma_start(w2_sb[:], w2[:])
            nc.scalar.dma_start(b1_sb[:], b1[:])
            nc.scalar.dma_start(b2_sb[:], b2[:])
            nc.vector.memset(cs_w[:], WS)

            # persistent activations
            kt_sb = singles.tile([P, DC, N], FP8)       # kT, d on partitions
            qt_sb = singles.tile([P, DC, NB], FP8)      # qT
            v_sb = singles.tile([P, NJC, D], FP8)       # v, j on partitions
            ctxacc = singles.tile([P, NIH, DC, IH], F32)  # partial ctxU sums
            denacc = singles.tile([1, NIH, IH], F32)      # partial den sums
            out_sb = singles.tile([1, NB], F32)

            # collective buffers
            bounce = dram.tile([P, BCOLS], FP8)
            agout = dram.tile([NCORES, P, BCOLS], FP8, addr_space="Shared")

            with (
                tc.tile_pool(name="xtiles", bufs=3) as xtiles,
                tc.tile_pool(name="shr", bufs=2) as shr,
                tc.tile_pool(name="etile", bufs=6) as etile,
                tc.tile_pool(name="mlp", bufs=2) as mlp,
                tc.tile_pool(name="ps_mm", bufs=3, space="PSUM") as ps_mm,
                tc.tile_pool(name="ps_ctx", bufs=1, space="PSUM") as ps_ctx,
                tc.tile_pool(name="ps_cs", bufs=1, space="PSUM") as ps_cs,
            ):
                # ---- phase 0: own-block kT/v shares -> bounce, then qT ----
                for it, xq_t in enumerate([xq_t0, xq_t1]):
                    skt_t = shr.tile([P, DC, JT], FP8, tag="skt")
                    for dc in range(DC):
                        ps = ps_mm.tile([P, JT], F32, tag="st")
                        for kp in range(KP):
                            nc.tensor.matmul(
                                ps[:],
                                wk_sb[:, 2 * kp:2 * kp + 2, dc * P:(dc + 1) * P],
                                xq_t[:, 2 * kp:2 * kp + 2],
                                start=(kp == 0), stop=(kp == KP - 1),
                                perf_mode=DR)
                        nc.vector.tensor_scalar_add(
                            skt_t[:, dc], ps[:], bk_sb[:, dc:dc + 1])
                    for dc in range(DC):
                        nc.sync.dma_start(
                            bounce[:, dc * NB + it * JT:dc * NB + it * JT + JT],
                            skt_t[:, dc])
                    sv_t = shr.tile([P, 4, D], FP8, tag="sv")
                    for jc in range(4):
                        ps = ps_mm.tile([P, D], F32, tag="st")
                        for kp in range(KP):
                            nc.tensor.matmul(
                                ps[:],
                                xq_t[:, 2 * kp:2 * kp + 2, jc * P:(jc + 1) * P],
                                wv_sb[:, 2 * kp:2 * kp + 2],
                                start=(kp == 0), stop=(kp == KP - 1),
                                perf_mode=DR)
                        nc.vector.tensor_tensor(
                            sv_t[:, jc], ps[:], bv_sb[:], mybir.AluOpType.add)
                    nc.sync.dma_start(
                        bounce[:, DC * NB + it * 4 * D:DC * NB + (it + 1) * 4 * D],
                        sv_t[:])

                # ---- AllGather of the fp8 shares (runs on TOPSP/SDMA;
                # overlaps all PE work below) ----
                nc.gpsimd.collective_compute(
                    "AllGather",
                    mybir.AluOpType.bypass,
                    replica_groups=[list(range(NCORES))],
                    ins=[bounce[:].opt()],
                    outs=[agout[:].opt()],
                )

                # qT for the own block (reuses the share's xq tiles)
                for it, xq_t in enumerate([xq_t0, xq_t1]):
                    for dc in range(DC):
                        ps = ps_mm.tile([P, JT], F32, tag="st")
                        for kp in range(KP):
                            nc.tensor.matmul(
                                ps[:],
                                wq_sb[:, 2 * kp:2 * kp + 2, dc * P:(dc + 1) * P],
                                xq_t[:, 2 * kp:2 * kp + 2],
                                start=(kp == 0), stop=(kp == KP - 1),
                                perf_mode=DR)
                        nc.vector.tensor_scalar_add(
                            qt_sb[:, dc, it * JT:(it + 1) * JT], ps[:],
                            bq_sb[:, dc:dc + 1])

                # ---- phase 1: replicated prefix blocks 0..MPRE-1 ----
                for jt in range(2 * MPRE):
                    xp_t = xtiles.tile([P, KC, JT], FP8, tag="xt")
                    nc.sync.dma_start(xp_t[:], xp[:, :, jt * JT:(jt + 1) * JT])
                    for dc in range(DC):
                        ps = ps_mm.tile([P, JT], F32, tag="st")
                        for kp in range(KP):
                            nc.tensor.matmul(
                                ps[:],
                                wk_sb[:, 2 * kp:2 * kp + 2, dc * P:(dc + 1) * P],
                                xp_t[:, 2 * kp:2 * kp + 2],
                                start=(kp == 0), stop=(kp == KP - 1),
                                perf_mode=DR)
                        nc.vector.tensor_scalar_add(
                            kt_sb[:, dc, jt * JT:(jt + 1) * JT], ps[:],
                            bk_sb[:, dc:dc + 1])
                    for jc in range(JT // P):
                        ps = ps_mm.tile([P, D], F32, tag="st")
                        for kp in range(KP):
                            nc.tensor.matmul(
                                ps[:],
                                xp_t[:, 2 * kp:2 * kp + 2, jc * P:(jc + 1) * P],
                                wv_sb[:, 2 * kp:2 * kp + 2],
                                start=(kp == 0), stop=(kp == KP - 1),
                                perf_mode=DR)
                        nc.vector.tensor_tensor(
                            v_sb[:, jt * (JT // P) + jc, :], ps[:], bv_sb[:],
                            mybir.AluOpType.add)

                # ---- unload gathered blocks MPRE..7 into kt_sb / v_sb ----
                # only sync/gpsimd here: their queues have no phase-2 work,
                # so blocking on the collective's semaphore is harmless.
                # kT on sync, v on gpsimd: block MPRE's kT and v land in
                # parallel right after the AllGather completes.
                for r in range(MPRE, NCORES):
                    nc.sync.dma_start(
                        kt_sb[:, :, r * NB:(r + 1) * NB],
                        agout[r, :, 0:DC * NB].rearrange(
                            "p (d n) -> p d n", d=DC))
                    nc.gpsimd.dma_start(
                        out=v_sb[:, 8 * r:8 * r + 8, :],
                        in_=agout[r, :, DC * NB:BCOLS].rearrange(
                            "p (c d) -> p c d", c=8))

                # ---- phase 2: attention in 4 superchunks; each final
                # chunk's PE epilogue (W1/W2 matmuls) is emitted AFTER the
                # next chunk's first attention units so the tensor queue
                # never stalls on the DVE evacuation/reciprocal chain ----
                def attn_unit(t, i0, ctx_ps, cs_ps, first, last):
                    e_t = etile.tile([P, 2, IH], FP8, tag="et",
                                     name=f"e_{i0}_{t}")
                    for s in range(2):
                        jc = 2 * t + s
                        st_ps = ps_mm.tile([P, IH], F32, tag="st",
                                           name=f"st_{i0}_{t}_{s}")
                        for dp in range(DP):
                            nc.tensor.matmul(
                                st_ps[:],
                                kt_sb[:, 2 * dp:2 * dp + 2,
                                      jc * P:(jc + 1) * P],
                                qt_sb[:, 2 * dp:2 * dp + 2, i0:i0 + IH],
                                start=(dp == 0), stop=(dp == DP - 1),
                                perf_mode=DR)
                        nc.scalar.activation(
                            e_t[:, s], st_ps[:],
                            mybir.ActivationFunctionType.Exp,
                            bias=0.0, scale=SCALE)
                    nc.tensor.matmul(cs_ps[:], cs_w[:], e_t[:],
                                     start=first, stop=last, perf_mode=DR)
                    for dc in range(DC):
                        nc.tensor.matmul(
                            ctx_ps[:, dc],
                            v_sb[:, 2 * t:2 * t + 2, dc * P:(dc + 1) * P],
                            e_t[:],
                            start=first, stop=last, perf_mode=DR)

                def epi_dve(ih, ctx_ps, cs_ps):
                    # total sums + reciprocal; frees PSUM for the next chunk
                    ctxu = mlp.tile([P, DC, IH], BF16, tag="ctxu",
                                    name=f"ctxu_{ih}")
                    for dc in range(DC):
                        nc.vector.tensor_tensor(ctxu[:, dc], ctx_ps[:, dc],
                                                ctxacc[:, ih, dc],
                                                mybir.AluOpType.add)
                    dent = mlp.tile([1, IH], F32, tag="dent",
                                    name=f"dent_{ih}")
                    nc.vector.tensor_tensor(dent[:], cs_ps[0:1],
                                            denacc[:, ih],
                                            mybir.AluOpType.add)
                    recip_sb = mlp.tile([1, IH], F32, tag="recip",
                                        name=f"recip_{ih}")
                    nc.vector.reciprocal(recip_sb[:], dent[:])
                    r_sb = mlp.tile([P, IH], F32, tag="rsb",
                                    name=f"rsb_{ih}")
                    nc.gpsimd.partition_broadcast(r_sb[:], recip_sb[:])
                    return ctxu, r_sb

                def epi_pe(ih, ctxu, r_sb):
                    # g = W1^T ctxU;  h = relu(g/den + b1);  out = W2^T h
                    i0 = ih * IH
                    sc_ps = ps_mm.tile([1, IH], F32, tag="st",
                                       name=f"sc_{ih}")
                    h_sb = mlp.tile([P, MC, IH], BF16, tag="hsb",
                                    name=f"h_{ih}")
                    for mc in range(MC):
                        g_ps = ps_mm.tile([P, IH], F32, tag="st",
                                          name=f"g_{ih}_{mc}")
                        for dc in range(DC):
                            nc.tensor.matmul(g_ps[:], w1_sb[:, dc, mc],
                                             ctxu[:, dc],
                                             start=(dc == 0),
                                             stop=(dc == DC - 1))
                        gsc = mlp.tile([P, IH], BF16, tag="gsc",
                                       name=f"gsc_{ih}_{mc}")
                        nc.vector.tensor_tensor(gsc[:], g_ps[:], r_sb[:],
                                                mybir.AluOpType.mult)
                        nc.scalar.activation(
                            h_sb[:, mc], gsc[:],
                            mybir.ActivationFunctionType.Relu,
                            bias=b1_sb[:, mc:mc + 1], scale=1.0)
                    for mc in range(MC):
                        nc.tensor.matmul(sc_ps[:], w2_sb[:, mc:mc + 1],
                                         h_sb[:, mc],
                                         start=(mc == 0), stop=(mc == MC - 1))
                    nc.scalar.add(out_sb[:, i0:i0 + IH], sc_ps[:], b2_sb[:])

                pending = None
                for ih, blo, bhi in ((0, 0, MPRE), (1, 0, MPRE),
                                     (0, MPRE, NCORES), (1, MPRE, NCORES)):
                    i0 = ih * IH
                    ctx_ps = ps_ctx.tile([P, DC, IH], F32, tag="ctx",
                                         name=f"ctx_{ih}_{blo}")
                    cs_ps = ps_cs.tile([32, IH], F32, tag="cs",
                                       name=f"cs_{ih}_{blo}")
                    t_first, t_last = 4 * blo, 4 * bhi - 1
                    for t in range(t_first, t_last + 1):
                        attn_unit(t, i0, ctx_ps, cs_ps,
                                  t == t_first, t == t_last)
                        if pending is not None and t == t_first + 1:
                            epi_pe(*pending)
                            pending = None
                    if bhi < NCORES:
                        # stash partial sums; frees PSUM for the next chunk
                        for dc in range(DC):
                            nc.vector.tensor_copy(ctxacc[:, ih, dc],
                                                  ctx_ps[:, dc])
                        nc.vector.tensor_copy(denacc[:, ih], cs_ps[0:1])
                    else:
                        pending = (ih,) + epi_dve(ih, ctx_ps, cs_ps)
                epi_pe(*pending)

            nc.sync.dma_start(out[:], out_sb[:])

    nc.finalize()
    return nc


def _prep(inputs):
    """Host-side layout prep shared by all cores + per-core xq blocks."""
    f32 = np.float32
    bf16 = ml_dtypes.bfloat16
    fp8 = ml_dtypes.float8_e4m3
    x = np.ascontiguousarray(inputs["x"], dtype=f32)
    xt = np.ascontiguousarray(x.T)                                   # [IN, N]
    xt_r = np.ascontiguousarray(
        xt.reshape(KC, P, N).transpose(1, 0, 2).astype(fp8))         # [P, KC, N]

    def w_r(w):  # [IN, D] -> [P, KC, D], x8 scale into fp8 range
        return np.ascontiguousarray(
            (np.asarray(w, f32) * WS).reshape(KC, P, D)
            .transpose(1, 0, 2).astype(fp8))

    w1_r = np.ascontiguousarray(
        np.asarray(inputs["W1"], f32).reshape(DC, P, MC, P)
        .transpose(1, 0, 2, 3).astype(bf16))                         # [P, DC, MC, P]
    w2_r = np.ascontiguousarray(
        np.asarray(inputs["W2"], f32).reshape(MC, P).T.astype(bf16))  # [P, MC]

    def b_col(b, nchunks, scale=1.0):  # [nchunks*P] -> [P, nchunks]
        return np.ascontiguousarray(
            (np.asarray(b, f32) * scale).reshape(nchunks, P).T)

    shared = {
        "xp": np.ascontiguousarray(xt_r[:, :, 0:MPRE * NB]),
        "wq": w_r(inputs["Wq"]),
        "wk": w_r(inputs["Wk"]),
        "wv": w_r(inputs["Wv"]),
        "w1": w1_r,
        "w2": w2_r,
        "bq": b_col(inputs["bq"], DC, WS),
        "bk": b_col(inputs["bk"], DC, WS),
        "bv": np.ascontiguousarray(
            np.broadcast_to(np.asarray(inputs["bv"], f32) * WS, (P, D))),
        "b1": b_col(inputs["b1"], MC),
        "b2": np.asarray(inputs["b2"], f32).reshape(1, 1),
    }
    xqs = [np.ascontiguousarray(xt_r[:, :, c * NB:(c + 1) * NB])
           for c in range(NCORES)]
    return shared, xqs


def kernel(**inputs) -> np.ndarray:
    if "nc" not in _CACHE:
        _CACHE["nc"] = _build()
    nc = _CACHE["nc"]
    shared, xqs = _prep(inputs)
    in_maps = [dict(shared, xq=xqs[c]) for c in range(NCORES)]
    res = run_bass_kernel_spmd(nc, in_maps, core_ids=list(range(NCORES)))
    return np.concatenate([res.results[c]["out"][0] for c in range(NCORES)])


# revision 14
# speedup vs baseline: 1.0278x; 1.0278x over previous
"""CAAN attention-scorer kernel for 8 Trainium2 NeuronCores.

scores = relu(softmax(QK^T/sqrt(D)) @ V @ W1 + b1) @ W2 + b2
with Q/K/V = x @ W{q,k,v} + b{q,k,v};  N=8192, IN_DIM=1024, D=512.

Row-parallel attention with SHARDED K/V projections + AllGather:

  - Core c computes kT/v only for its own j-block (1/8 of N) and
    AllGathers the fp8 results (1 MB in -> 8 MB out).  This removes the
    448 replicated DoubleRow matmul slots (~100 us) the old kernel spent
    projecting all of K/V on every core.
  - The collective's latency (~40 us barrier/skew + ~40 us data) is
    hidden behind a small replicated PREFIX: every core also projects
    blocks 0..MPRE-1 locally (consumed first in phase 2, both i-halves,
    before any gathered data is touched).  Caveat: any collective trips
    a sticky GPIO clock-throttle (PE capped at 13/16 for the rest of the
    run, ~268 ns/matmul vs ~229); the sharding still nets ~30 us.
  - Phase 2 runs in 4 superchunks (ih0/ih1 x prefix/rest) accumulating
    partial ctx/den sums into SBUF f32 so both i-halves' prefix work can
    run before the AllGather lands (PSUM can't hold both halves at once).
  - Softmax normalization is deferred THROUGH the MLP's first matmul:
    g = W1^T ctxU (unnormalized), h = relu(g * recip + b1).  The
    reciprocal + gpsimd partition-broadcast run off the PE critical path.

Everything runs in a transposed layout so softmax denominators come from a
constant-column PE matmul:

  kT[d, j]   = (8 Wk)^T x^T (+8 bk)   d on partitions   (fp8, x8 scale)
  qT[d, i]   = (8 Wq)^T xq^T (+8 bq)                    (fp8, x8 scale)
  v[j, d]    = x (8 Wv) (+8 bv)       j on partitions   (fp8)
  S^T[j, i]  = kT-chunk^T qT          (= 64 * true scores)
  E          = exp(S^T / (64 sqrt(D)))     PSUM -> SBUF fp8
  ctxU^T    += v-chunk^T E            (= 8 * true ctxU)
  den[1,i]  += (8.0)^T E              (= 8 * true denom -> cancels the x8)
  g^T[m, i]  = W1-chunk^T ctxU^T      (bf16)
  h^T        = relu(g^T * (1/den) + b1)
  out[1, i]  = W2^T h^T + b2
"""

import numpy as np
import ml_dtypes

import concourse.tile as tile
from concourse import bacc, mybir
from concourse.bass_utils import run_bass_kernel_spmd

N, IN_DIM, D = 8192, 1024, 512
NCORES = 8
NB = N // NCORES            # 1024 rows per core
P = 128
KC = IN_DIM // P            # 8  k-chunks of the input dim
KP = KC // 2                # 4  DoubleRow k-pairs
DC = D // P                 # 4  d-chunks
DP = DC // 2                # 2  DoubleRow d-pairs
MC = (D // 2) // P          # 2  m-chunks of the hidden dim
JT = 512                    # j-tile width in phase 1
NJC = N // P                # 64 j-chunks
NJP = NJC // 2              # 32 j-pairs in phase 2
IH = 512                    # i-half width in phase 2
NIH = NB // IH              # 2
MPRE = 2                    # replicated prefix blocks (hide the AllGather)
WS = 8.0                    # fp8 weight pre-scale (exact power of two)
SCALE = 1.0 / float(np.sqrt(np.float32(D))) / (WS * WS)
BCOLS = DC * NB + 8 * D     # 8192 bounce cols: [kT share | v share]

FP8 = mybir.dt.float8e4
BF16 = mybir.dt.bfloat16
F32 = mybir.dt.float32
DR = mybir.MatmulPerfMode.DoubleRow

_CACHE = {}


def _build():
    nc = bacc.Bacc(None, target_bir_lowering=False, debug=False,
                   num_devices=NCORES)

    xq = nc.declare_dram_parameter("xq", [P, KC, NB], FP8, isOutput=False)
    xp = nc.declare_dram_parameter("xp", [P, KC, MPRE * NB], FP8, isOutput=False)
    wq = nc.declare_dram_parameter("wq", [P, KC, D], FP8, isOutput=False)
    wk = nc.declare_dram_parameter("wk", [P, KC, D], FP8, isOutput=False)
    wv = nc.declare_dram_parameter("wv", [P, KC, D], FP8, isOutput=False)
    w1 = nc.declare_dram_parameter("w1", [P, DC, MC, P], BF16, isOutput=False)
    w2 = nc.declare_dram_parameter("w2", [P, MC], BF16, isOutput=False)
    bq = nc.declare_dram_parameter("bq", [P, DC], F32, isOutput=False)   # x8
    bk = nc.declare_dram_parameter("bk", [P, DC], F32, isOutput=False)   # x8
    bv = nc.declare_dram_parameter("bv", [P, D], F32, isOutput=False)    # x8
    b1 = nc.declare_dram_parameter("b1", [P, MC], F32, isOutput=False)
    b2 = nc.declare_dram_parameter("b2", [1, 1], F32, isOutput=False)
    out = nc.declare_dram_parameter("out", [1, NB], F32, isOutput=True)

    with tile.TileContext(nc) as tc:
        with (
            tc.tile_pool(name="singles", bufs=1) as singles,
            tc.tile_pool(name="dram", bufs=1, space="DRAM") as dram,
        ):
            # ---- constants / weights into SBUF ----
            wq_sb = singles.tile([P, KC, D], FP8)
            wk_sb = singles.tile([P, KC, D], FP8)
            wv_sb = singles.tile([P, KC, D], FP8)
            w1_sb = singles.tile([P, DC, MC, P], BF16)
            w2_sb = singles.tile([P, MC], BF16)
            bq_sb = singles.tile([P, DC], F32)
            bk_sb = singles.tile([P, DC], F32)
            bv_sb = singles.tile([P, D], F32)
            b1_sb = singles.tile([P, MC], F32)
            b2_sb = singles.tile([1, 1], F32)
            cs_w = singles.tile([P, 2, 32], FP8)     # colsum weights = 8.0
            # (dual-fp8 ldweights needs >=32 active columns; rows identical)

            # critical-path DMAs first, spread across idle engine queues so
            # the first share matmul can start ~3us after the preamble
            xq_t0 = singles.tile([P, KC, JT], FP8)
            xq_t1 = singles.tile([P, KC, JT], FP8)
            nc.sync.dma_start(wk_sb[:, 0:2], wk[:, 0:2])
            nc.scalar.dma_start(xq_t0[:, 0:2], xq[:, 0:2, 0:JT])
            for kp in range(1, KP):
                s = slice(2 * kp, 2 * kp + 2)
                nc.scalar.dma_start(wk_sb[:, s], wk[:, s])
                nc.sync.dma_start(xq_t0[:, s], xq[:, s, 0:JT])
            nc.scalar.dma_start(bk_sb[:], bk[:])
            nc.scalar.dma_start(bv_sb[:], bv[:])
            for kp in range(KP):
                s = slice(2 * kp, 2 * kp + 2)
                eng = nc.sync if kp % 2 == 0 else nc.scalar
                eng.dma_start(xq_t1[:, s], xq[:, s, JT:2 * JT])
            nc.gpsimd.dma_start(out=wv_sb[:], in_=wv[:])
            nc.gpsimd.dma_start(out=wq_sb[:], in_=wq[:])
            nc.scalar.dma_start(bq_sb[:], bq[:])
            nc.scalar.dma_start(w1_sb[:], w1[:])
            nc.scalar.dma_start(w2_sb[:], w2[:])
            nc.scalar.dma_start(b1_sb[:], b1[:])
            nc.scalar.dma_start(b2_sb[:], b2[:])
            nc.vector.memset(cs_w[:], WS)

            # persistent activations
            kt_sb = singles.tile([P, DC, N], FP8)       # kT, d on partitions
            qt_sb = singles.tile([P, DC, NB], FP8)      # qT
            v_sb = singles.tile([P, NJC, D], FP8)       # v, j on partitions
            ctxacc = singles.tile([P, NIH, DC, IH], F32)  # partial ctxU sums
            denacc = singles.tile([1, NIH, IH], F32)      # partial den sums
            out_sb = singles.tile([1, NB], F32)

            # collective buffers
            bounce = dram.tile([P, BCOLS], FP8)
            agout = dram.tile([NCORES, P, BCOLS], FP8, addr_space="Shared")

            with (
                tc.tile_pool(name="xtiles", bufs=3) as xtiles,
                tc.tile_pool(name="shr", bufs=2) as shr,
                tc.tile_pool(name="etile", bufs=6) as etile,
                tc.tile_pool(name="mlp", bufs=2) as mlp,
                tc.tile_pool(name="ps_mm", bufs=3, space="PSUM") as ps_mm,
                tc.tile_pool(name="ps_ctx", bufs=1, space="PSUM") as ps_ctx,
                tc.tile_pool(name="ps_cs", bufs=1, space="PSUM") as ps_cs,
            ):
                # ---- phase 0: own-block kT/v shares -> bounce, then qT ----
                for it, xq_t in enumerate([xq_t0, xq_t1]):
                    skt_t = shr.tile([P, DC, JT], FP8, tag="skt")
                    for dc in range(DC):
                        ps = ps_mm.tile([P, JT], F32, tag="st")
                        for kp in range(KP):
                            nc.tensor.matmul(
                                ps[:],
                                wk_sb[:, 2 * kp:2 * kp + 2, dc * P:(dc + 1) * P],
                                xq_t[:, 2 * kp:2 * kp + 2],
                                start=(kp == 0), stop=(kp == KP - 1),
                                perf_mode=DR)
                        nc.vector.tensor_scalar_add(
                            skt_t[:, dc], ps[:], bk_sb[:, dc:dc + 1])
                    for dc in range(DC):
                        nc.sync.dma_start(
                            bounce[:, dc * NB + it * JT:dc * NB + it * JT + JT],
                            skt_t[:, dc])
                    sv_t = shr.tile([P, 4, D], FP8, tag="sv")
                    for jc in range(4):
                        ps = ps_mm.tile([P, D], F32, tag="st")
                        for kp in range(KP):
                            nc.tensor.matmul(
                                ps[:],
                                xq_t[:, 2 * kp:2 * kp + 2, jc * P:(jc + 1) * P],
                                wv_sb[:, 2 * kp:2 * kp + 2],
                                start=(kp == 0), stop=(kp == KP - 1),
                                perf_mode=DR)
                        nc.vector.tensor_tensor(
                            sv_t[:, jc], ps[:], bv_sb[:], mybir.AluOpType.add)
                    nc.sync.dma_start(
                        bounce[:, DC * NB + it * 4 * D:DC * NB + (it + 1) * 4 * D],
                        sv_t[:])

                # ---- AllGather of the fp8 shares (runs on TOPSP/SDMA;
                # overlaps all PE work below) ----
                nc.gpsimd.collective_compute(
                    "AllGather",
                    mybir.AluOpType.bypass,
                    replica_groups=[list(range(NCORES))],
                    ins=[bounce[:].opt()],
                    outs=[agout[:].opt()],
                )

                # qT for the own block (reuses the share's xq tiles)
                for it, xq_t in enumerate([xq_t0, xq_t1]):
                    for dc in range(DC):
                        ps = ps_mm.tile([P, JT], F32, tag="st")
                        for kp in range(KP):
                            nc.tensor.matmul(
                                ps[:],
                                wq_sb[:, 2 * kp:2 * kp + 2, dc * P:(dc + 1) * P],
                                xq_t[:, 2 * kp:2 * kp + 2],
                                start=(kp == 0), stop=(kp == KP - 1),
                                perf_mode=DR)
                        nc.vector.tensor_scalar_add(
                            qt_sb[:, dc, it * JT:(it + 1) * JT], ps[:],
                            bq_sb[:, dc:dc + 1])

                # ---- phase 1: replicated prefix blocks 0..MPRE-1 ----
                for jt in range(2 * MPRE):
                    xp_t = xtiles.tile([P, KC, JT], FP8, tag="xt")
                    nc.sync.dma_start(xp_t[:], xp[:, :, jt * JT:(jt + 1) * JT])
                    for dc in range(DC):
                        ps = ps_mm.tile([P, JT], F32, tag="st")
                        for kp in range(KP):
                            nc.tensor.matmul(
                                ps[:],
                                wk_sb[:, 2 * kp:2 * kp + 2, dc * P:(dc + 1) * P],
                                xp_t[:, 2 * kp:2 * kp + 2],
                                start=(kp == 0), stop=(kp == KP - 1),
                                perf_mode=DR)
                        nc.vector.tensor_scalar_add(
                            kt_sb[:, dc, jt * JT:(jt + 1) * JT], ps[:],
                            bk_sb[:, dc:dc + 1])
                    for jc in range(JT // P):
                        ps = ps_mm.tile([P, D], F32, tag="st")
                        for kp in range(KP):
                            nc.tensor.matmul(
                                ps[:],
                                xp_t[:, 2 * kp:2 * kp + 2, jc * P:(jc + 1) * P],
                                wv_sb[:, 2 * kp:2 * kp + 2],
                                start=(kp == 0), stop=(kp == KP - 1),
                                perf_mode=DR)
                        nc.vector.tensor_tensor(
                            v_sb[:, jt * (JT // P) + jc, :], ps[:], bv_sb[:],
                            mybir.AluOpType.add)

                # ---- unload gathered blocks MPRE..7 into kt_sb / v_sb ----
                # only sync/gpsimd here: their queues have no phase-2 work,
                # so blocking on the collective's semaphore is harmless.
                # kT on sync, v on gpsimd: block MPRE's kT and v land in
                # parallel right after the AllGather completes.
                for r in range(MPRE, NCORES):
                    nc.sync.dma_start(
                        kt_sb[:, :, r * NB:(r + 1) * NB],
                        agout[r, :, 0:DC * NB].rearrange(
                            "p (d n) -> p d n", d=DC))
                    nc.gpsimd.dma_start(
                        out=v_sb[:, 8 * r:8 * r + 8, :],
                        in_=agout[r, :, DC * NB:BCOLS].rearrange(
                            "p (c d) -> p c d", c=8))

                # ---- phase 2: attention in 4 superchunks; each final
                # chunk's PE epilogue (W1/W2 matmuls) is emitted AFTER the
                # next chunk's first attention units so the tensor queue
                # never stalls on the DVE evacuation/reciprocal chain ----
                def attn_unit(t, i0, ctx_ps, cs_ps, first, last):
                    e_t = etile.tile([P, 2, IH], FP8, tag="et",
                                     name=f"e_{i0}_{t}")
                    for s in range(2):
                        jc = 2 * t + s
                        st_ps = ps_mm.tile([P, IH], F32, tag="st",
                                           name=f"st_{i0}_{t}_{s}")
                        for dp in range(DP):
                            nc.tensor.matmul(
                                st_ps[:],
                                kt_sb[:, 2 * dp:2 * dp + 2,
                                      jc * P:(jc + 1) * P],
                                qt_sb[:, 2 * dp:2 * dp + 2, i0:i0 + IH],
                                start=(dp == 0), stop=(dp == DP - 1),
                                perf_mode=DR)
                        nc.scalar.activation(
                            e_t[:, s], st_ps[:],
                            mybir.ActivationFunctionType.Exp,
                            bias=0.0, scale=SCALE)
                    nc.tensor.matmul(cs_ps[:], cs_w[:], e_t[:],
                                     start=first, stop=last, perf_mode=DR)
                    for dc in range(DC):
                        nc.tensor.matmul(
                            ctx_ps[:, dc],
                            v_sb[:, 2 * t:2 * t + 2, dc * P:(dc + 1) * P],
                            e_t[:],
                            start=first, stop=last, perf_mode=DR)

                def epi_dve(ih, ctx_ps, cs_ps):
                    # total sums + reciprocal; frees PSUM for the next chunk
                    ctxu = mlp.tile([P, DC, IH], BF16, tag="ctxu",
                                    name=f"ctxu_{ih}")
                    for dc in range(DC):
                        nc.vector.tensor_tensor(ctxu[:, dc], ctx_ps[:, dc],
                                                ctxacc[:, ih, dc],
                                                mybir.AluOpType.add)
                    dent = mlp.tile([1, IH], F32, tag="dent",
                                    name=f"dent_{ih}")
                    nc.vector.tensor_tensor(dent[:], cs_ps[0:1],
                                            denacc[:, ih],
                                            mybir.AluOpType.add)
                    recip_sb = mlp.tile([1, IH], F32, tag="recip",
                                        name=f"recip_{ih}")
                    nc.vector.reciprocal(recip_sb[:], dent[:])
                    r_sb = mlp.tile([P, IH], F32, tag="rsb",
                                    name=f"rsb_{ih}")
                    nc.gpsimd.partition_broadcast(r_sb[:], recip_sb[:])
                    return ctxu, r_sb

                def epi_pe(ih, ctxu, r_sb):
                    # g = W1^T ctxU;  h = relu(g/den + b1);  out = W2^T h
                    i0 = ih * IH
                    sc_ps = ps_mm.tile([1, IH], F32, tag="st",
                                       name=f"sc_{ih}")
                    h_sb = mlp.tile([P, MC, IH], BF16, tag="hsb",
                                    name=f"h_{ih}")
                    for mc in range(MC):
                        g_ps = ps_mm.tile([P, IH], F32, tag="st",
                                          name=f"g_{ih}_{mc}")
                        for dc in range(DC):
                            nc.tensor.matmul(g_ps[:], w1_sb[:, dc, mc],
                                             ctxu[:, dc],
                                             start=(dc == 0),
                                             stop=(dc == DC - 1))
                        gsc = mlp.tile([P, IH], BF16, tag="gsc",
                                       name=f"gsc_{ih}_{mc}")
                        nc.vector.tensor_tensor(gsc[:], g_ps[:], r_sb[:],
                                                mybir.AluOpType.mult)
                        nc.scalar.activation(
                            h_sb[:, mc], gsc[:],
                            mybir.ActivationFunctionType.Relu,
                            bias=b1_sb[:, mc:mc + 1], scale=1.0)
                    for mc in range(MC):
                        nc.tensor.matmul(sc_ps[:], w2_sb[:, mc:mc + 1],
                                         h_sb[:, mc],
                                         start=(mc == 0), stop=(mc == MC - 1))
                    nc.scalar.add(out_sb[:, i0:i0 + IH], sc_ps[:], b2_sb[:])

                pending = None
                for ih, blo, bhi in ((0, 0, MPRE), (1, 0, MPRE),
                                     (0, MPRE, NCORES), (1, MPRE, NCORES)):
                    i0 = ih * IH
                    ctx_ps = ps_ctx.tile([P, DC, IH], F32, tag="ctx",
                                         name=f"ctx_{ih}_{blo}")
                    cs_ps = ps_cs.tile([32, IH], F32, tag="cs",
                                       name=f"cs_{ih}_{blo}")
                    t_first, t_last = 4 * blo, 4 * bhi - 1
                    for t in range(t_first, t_last + 1):
                        attn_unit(t, i0, ctx_ps, cs_ps,
                                  t == t_first, t == t_last)
                        if pending is not None and t == t_first + 1:
                            epi_pe(*pending)
                            pending = None
                    if bhi < NCORES:
                        # stash partial sums; frees PSUM for the next chunk
                        for dc in range(DC):
                            nc.vector.tensor_copy(ctxacc[:, ih, dc],
                                                  ctx_ps[:, dc])
                        nc.vector.tensor_copy(denacc[:, ih], cs_ps[0:1])
                    else:
                        pending = (ih,) + epi_dve(ih, ctx_ps, cs_ps)
                epi_pe(*pending)

            nc.sync.dma_start(out[:], out_sb[:])

    nc.finalize()
    return nc


def _prep(inputs):
    """Host-side layout prep shared by all cores + per-core xq blocks."""
    f32 = np.float32
    bf16 = ml_dtypes.bfloat16
    fp8 = ml_dtypes.float8_e4m3
    x = np.ascontiguousarray(inputs["x"], dtype=f32)
    xt = np.ascontiguousarray(x.T)                                   # [IN, N]
    xt_r = np.ascontiguousarray(
        xt.reshape(KC, P, N).transpose(1, 0, 2).astype(fp8))         # [P, KC, N]

    def w_r(w):  # [IN, D] -> [P, KC, D], x8 scale into fp8 range
        return np.ascontiguousarray(
            (np.asarray(w, f32) * WS).reshape(KC, P, D)
            .transpose(1, 0, 2).astype(fp8))

    w1_r = np.ascontiguousarray(
        np.asarray(inputs["W1"], f32).reshape(DC, P, MC, P)
        .transpose(1, 0, 2, 3).astype(bf16))                         # [P, DC, MC, P]
    w2_r = np.ascontiguousarray(
        np.asarray(inputs["W2"], f32).reshape(MC, P).T.astype(bf16))  # [P, MC]

    def b_col(b, nchunks, scale=1.0):  # [nchunks*P] -> [P, nchunks]
        return np.ascontiguousarray(
            (np.asarray(b, f32) * scale).reshape(nchunks, P).T)

    shared = {
        "xp": np.ascontiguousarray(xt_r[:, :, 0:MPRE * NB]),
        "wq": w_r(inputs["Wq"]),
        "wk": w_r(inputs["Wk"]),
        "wv": w_r(inputs["Wv"]),
        "w1": w1_r,
        "w2": w2_r,
        "bq": b_col(inputs["bq"], DC, WS),
        "bk": b_col(inputs["bk"], DC, WS),
        "bv": np.ascontiguousarray(
            np.broadcast_to(np.asarray(inputs["bv"], f32) * WS, (P, D))),
        "b1": b_col(inputs["b1"], MC),
        "b2": np.asarray(inputs["b2"], f32).reshape(1, 1),
    }
    xqs = [np.ascontiguousarray(xt_r[:, :, c * NB:(c + 1) * NB])
           for c in range(NCORES)]
    return shared, xqs


def kernel(**inputs) -> np.ndarray:
    if "nc" not in _CACHE:
        _CACHE["nc"] = _build()
    nc = _CACHE["nc"]
    shared, xqs = _prep(inputs)
    in_maps = [dict(shared, xq=xqs[c]) for c in range(NCORES)]
    res = run_bass_kernel_spmd(nc, in_maps, core_ids=list(range(NCORES)))
    return np.concatenate([res.results[c]["out"][0] for c in range(NCORES)])


# revision 19
# speedup vs baseline: 1.0356x; 1.0076x over previous
"""CAAN attention-scorer kernel for 8 Trainium2 NeuronCores.

scores = relu(softmax(QK^T/sqrt(D)) @ V @ W1 + b1) @ W2 + b2
with Q/K/V = x @ W{q,k,v} + b{q,k,v};  N=8192, IN_DIM=1024, D=512.

Row-parallel attention with SHARDED K/V projections + AllGather:

  - Core c computes kT/v only for its own j-block (1/8 of N) and
    AllGathers the fp8 results (1 MB in -> 8 MB out).  This removes the
    448 replicated DoubleRow matmul slots (~100 us) the old kernel spent
    projecting all of K/V on every core.
  - The collective's latency (~40 us barrier/skew + ~40 us data) is
    hidden behind a small replicated PREFIX: every core also projects
    blocks 0..MPRE-1 locally (consumed first in phase 2, both i-halves,
    before any gathered data is touched).  Caveat: any collective trips
    a sticky GPIO clock-throttle (PE capped at 13/16 for the rest of the
    run, ~268 ns/matmul vs ~229); the sharding still nets ~30 us.
  - Phase 2 runs in 4 superchunks (ih0/ih1 x prefix/rest) accumulating
    partial ctx/den sums into SBUF f32 so both i-halves' prefix work can
    run before the AllGather lands (PSUM can't hold both halves at once).
  - Softmax normalization is deferred THROUGH the MLP's first matmul:
    g = W1^T ctxU (unnormalized), h = relu(g * recip + b1).  The
    reciprocal + gpsimd partition-broadcast run off the PE critical path.

Everything runs in a transposed layout so softmax denominators come from a
constant-column PE matmul:

  kT[d, j]   = (8 Wk)^T x^T (+8 bk)   d on partitions   (fp8, x8 scale)
  qT[d, i]   = (8 Wq)^T xq^T (+8 bq)                    (fp8, x8 scale)
  v[j, d]    = x (8 Wv) (+8 bv)       j on partitions   (fp8)
  S^T[j, i]  = kT-chunk^T qT          (= 64 * true scores)
  E          = exp(S^T / (64 sqrt(D)))     PSUM -> SBUF fp8
  ctxU^T    += v-chunk^T E            (= 8 * true ctxU)
  den[1,i]  += (8.0)^T E              (= 8 * true denom -> cancels the x8)
  g^T[m, i]  = W1-chunk^T ctxU^T      (bf16)
  h^T        = relu(g^T * (1/den) + b1)
  out[1, i]  = W2^T h^T + b2
"""

import numpy as np
import ml_dtypes

import concourse.tile as tile
from concourse import bacc, mybir
from concourse.bass_utils import run_bass_kernel_spmd

N, IN_DIM, D = 8192, 1024, 512
NCORES = 8
NB = N // NCORES            # 1024 rows per core
P = 128
KC = IN_DIM // P            # 8  k-chunks of the input dim
KP = KC // 2                # 4  DoubleRow k-pairs
DC = D // P                 # 4  d-chunks
DP = DC // 2                # 2  DoubleRow d-pairs
MC = (D // 2) // P          # 2  m-chunks of the hidden dim
JT = 512                    # j-tile width in phase 1
NJC = N // P                # 64 j-chunks
NJP = NJC // 2              # 32 j-pairs in phase 2
IH = 512                    # i-half width in phase 2
NIH = NB // IH              # 2
MPRE = 2                    # replicated prefix blocks (hide the AllGather)
WS = 8.0                    # fp8 weight pre-scale (exact power of two)
SCALE = 1.0 / float(np.sqrt(np.float32(D))) / (WS * WS)
BCOLS = DC * NB + 8 * D     # 8192 bounce cols: [kT share | v share]

FP8 = mybir.dt.float8e4
BF16 = mybir.dt.bfloat16
F32 = mybir.dt.float32
DR = mybir.MatmulPerfMode.DoubleRow

_CACHE = {}


def _build():
    nc = bacc.Bacc(None, target_bir_lowering=False, debug=False,
                   num_devices=NCORES)

    xq = nc.declare_dram_parameter("xq", [P, KC, NB], FP8, isOutput=False)
    xp = nc.declare_dram_parameter("xp", [P, KC, MPRE * NB], FP8, isOutput=False)
    wq = nc.declare_dram_parameter("wq", [P, KC, D], FP8, isOutput=False)
    wk = nc.declare_dram_parameter("wk", [P, KC, D], FP8, isOutput=False)
    wv = nc.declare_dram_parameter("wv", [P, KC, D], FP8, isOutput=False)
    w1 = nc.declare_dram_parameter("w1", [P, DC, MC, P], BF16, isOutput=False)
    w2 = nc.declare_dram_parameter("w2", [P, MC], BF16, isOutput=False)
    bq = nc.declare_dram_parameter("bq", [P, DC], F32, isOutput=False)   # x8
    bk = nc.declare_dram_parameter("bk", [P, DC], F32, isOutput=False)   # x8
    bv = nc.declare_dram_parameter("bv", [P, D], F32, isOutput=False)    # x8
    b1 = nc.declare_dram_parameter("b1", [P, MC], F32, isOutput=False)
    b2 = nc.declare_dram_parameter("b2", [1, 1], F32, isOutput=False)
    out = nc.declare_dram_parameter("out", [1, NB], F32, isOutput=True)

    with tile.TileContext(nc) as tc:
        with (
            tc.tile_pool(name="singles", bufs=1) as singles,
            tc.tile_pool(name="dram", bufs=1, space="DRAM") as dram,
        ):
            # ---- constants / weights into SBUF ----
            wq_sb = singles.tile([P, KC, D], FP8)
            wk_sb = singles.tile([P, KC, D], FP8)
            wv_sb = singles.tile([P, KC, D], FP8)
            w1_sb = singles.tile([P, DC, MC, P], BF16)
            w2_sb = singles.tile([P, MC], BF16)
            bq_sb = singles.tile([P, DC], F32)
            bk_sb = singles.tile([P, DC], F32)
            bv_sb = singles.tile([P, D], F32)
            b1_sb = singles.tile([P, MC], F32)
            b2_sb = singles.tile([1, 1], F32)
            e8_w = singles.tile([P, 1], BF16)        # den matmul weights = 8.0

            # critical-path DMAs first, spread across idle engine queues so
            # the first share matmul can start ~3us after the preamble
            xq_t0 = singles.tile([P, KC, JT], FP8)
            xq_t1 = singles.tile([P, KC, JT], FP8)
            nc.sync.dma_start(wk_sb[:, 0:2], wk[:, 0:2])
            nc.scalar.dma_start(xq_t0[:, 0:2], xq[:, 0:2, 0:JT])
            for kp in range(1, KP):
                s = slice(2 * kp, 2 * kp + 2)
                nc.scalar.dma_start(wk_sb[:, s], wk[:, s])
                nc.sync.dma_start(xq_t0[:, s], xq[:, s, 0:JT])
            nc.scalar.dma_start(bk_sb[:], bk[:])
            nc.scalar.dma_start(bv_sb[:], bv[:])
            for kp in range(KP):
                s = slice(2 * kp, 2 * kp + 2)
                eng = nc.sync if kp % 2 == 0 else nc.scalar
                eng.dma_start(xq_t1[:, s], xq[:, s, JT:2 * JT])
            nc.gpsimd.dma_start(out=wv_sb[:], in_=wv[:])
            nc.gpsimd.dma_start(out=wq_sb[:], in_=wq[:])
            nc.scalar.dma_start(bq_sb[:], bq[:])
            nc.scalar.dma_start(w1_sb[:], w1[:])
            nc.scalar.dma_start(w2_sb[:], w2[:])
            nc.scalar.dma_start(b1_sb[:], b1[:])
            nc.scalar.dma_start(b2_sb[:], b2[:])
            nc.vector.memset(e8_w[:], WS)

            # persistent activations
            kt_sb = singles.tile([P, DC, N], FP8)       # kT, d on partitions
            qt_sb = singles.tile([P, DC, NB], FP8)      # qT
            v_sb = singles.tile([P, NJC, D], FP8)       # v, j on partitions
            ctxacc = singles.tile([P, NIH, DC, IH], F32)  # partial ctxU sums
            # DVE ping-pong accumulators for the softmax denominators: the
            # per-t colsum matmul moves off the PE (64 DR slots saved); the
            # 128-partition reduction happens in 2 tiny bf16 matmuls per
            # i-half at superchunk end
            esum = singles.tile([P, NIH, 2, 2, IH], BF16)
            for ih in range(NIH):
                nc.vector.memset(esum[:, ih, 0], 0.0)
            out_sb = singles.tile([1, NB], F32)

            # collective buffers
            bounce = dram.tile([P, BCOLS], FP8)
            agout = dram.tile([NCORES, P, BCOLS], FP8, addr_space="Shared")

            with (
                tc.tile_pool(name="xtiles", bufs=3) as xtiles,
                tc.tile_pool(name="shr", bufs=2) as shr,
                tc.tile_pool(name="etile", bufs=6) as etile,
                tc.tile_pool(name="mlp", bufs=2) as mlp,
                tc.tile_pool(name="ps_mm", bufs=3, space="PSUM") as ps_mm,
                tc.tile_pool(name="ps_ctx", bufs=1, space="PSUM") as ps_ctx,
                tc.tile_pool(name="ps_den", bufs=1, space="PSUM") as ps_den,
            ):
                # ---- phase 0: own-block kT/v shares -> bounce, then qT ----
                for it, xq_t in enumerate([xq_t0, xq_t1]):
                    skt_t = shr.tile([P, DC, JT], FP8, tag="skt")
                    for dc in range(DC):
                        ps = ps_mm.tile([P, JT], F32, tag="st")
                        for kp in range(KP):
                            nc.tensor.matmul(
                                ps[:],
                                wk_sb[:, 2 * kp:2 * kp + 2, dc * P:(dc + 1) * P],
                                xq_t[:, 2 * kp:2 * kp + 2],
                                start=(kp == 0), stop=(kp == KP - 1),
                                perf_mode=DR)
                        nc.vector.tensor_scalar_add(
                            skt_t[:, dc], ps[:], bk_sb[:, dc:dc + 1])
                    for dc in range(DC):
                        nc.sync.dma_start(
                            bounce[:, dc * NB + it * JT:dc * NB + it * JT + JT],
                            skt_t[:, dc])
                    sv_t = shr.tile([P, 4, D], FP8, tag="sv")
                    for jc in range(4):
                        ps = ps_mm.tile([P, D], F32, tag="st")
                        for kp in range(KP):
                            nc.tensor.matmul(
                                ps[:],
                                xq_t[:, 2 * kp:2 * kp + 2, jc * P:(jc + 1) * P],
                                wv_sb[:, 2 * kp:2 * kp + 2],
                                start=(kp == 0), stop=(kp == KP - 1),
                                perf_mode=DR)
                        nc.vector.tensor_tensor(
                            sv_t[:, jc], ps[:], bv_sb[:], mybir.AluOpType.add)
                    nc.sync.dma_start(
                        bounce[:, DC * NB + it * 4 * D:DC * NB + (it + 1) * 4 * D],
                        sv_t[:])

                # ---- AllGather of the fp8 shares (runs on TOPSP/SDMA;
                # overlaps all PE work below) ----
                nc.gpsimd.collective_compute(
                    "AllGather",
                    mybir.AluOpType.bypass,
                    replica_groups=[list(range(NCORES))],
                    ins=[bounce[:].opt()],
                    outs=[agout[:].opt()],
                )

                # qT for the own block (reuses the share's xq tiles)
                for it, xq_t in enumerate([xq_t0, xq_t1]):
                    for dc in range(DC):
                        ps = ps_mm.tile([P, JT], F32, tag="st")
                        for kp in range(KP):
                            nc.tensor.matmul(
                                ps[:],
                                wq_sb[:, 2 * kp:2 * kp + 2, dc * P:(dc + 1) * P],
                                xq_t[:, 2 * kp:2 * kp + 2],
                                start=(kp == 0), stop=(kp == KP - 1),
                                perf_mode=DR)
                        nc.vector.tensor_scalar_add(
                            qt_sb[:, dc, it * JT:(it + 1) * JT], ps[:],
                            bq_sb[:, dc:dc + 1])

                # ---- phase 1: replicated prefix blocks 0..MPRE-1 ----
                for jt in range(2 * MPRE):
                    xp_t = xtiles.tile([P, KC, JT], FP8, tag="xt")
                    nc.sync.dma_start(xp_t[:], xp[:, :, jt * JT:(jt + 1) * JT])
                    for dc in range(DC):
                        ps = ps_mm.tile([P, JT], F32, tag="st")
                        for kp in range(KP):
                            nc.tensor.matmul(
                                ps[:],
                                wk_sb[:, 2 * kp:2 * kp + 2, dc * P:(dc + 1) * P],
                                xp_t[:, 2 * kp:2 * kp + 2],
                                start=(kp == 0), stop=(kp == KP - 1),
                                perf_mode=DR)
                        nc.vector.tensor_scalar_add(
                            kt_sb[:, dc, jt * JT:(jt + 1) * JT], ps[:],
                            bk_sb[:, dc:dc + 1])
                    for jc in range(JT // P):
                        ps = ps_mm.tile([P, D], F32, tag="st")
                        for kp in range(KP):
                            nc.tensor.matmul(
                                ps[:],
                                xp_t[:, 2 * kp:2 * kp + 2, jc * P:(jc + 1) * P],
                                wv_sb[:, 2 * kp:2 * kp + 2],
                                start=(kp == 0), stop=(kp == KP - 1),
                                perf_mode=DR)
                        nc.vector.tensor_tensor(
                            v_sb[:, jt * (JT // P) + jc, :], ps[:], bv_sb[:],
                            mybir.AluOpType.add)

                # ---- unload gathered blocks MPRE..7 into kt_sb / v_sb ----
                # only sync/gpsimd here: their queues have no phase-2 work,
                # so blocking on the collective's semaphore is harmless.
                # kT on sync, v on gpsimd: block MPRE's kT and v land in
                # parallel right after the AllGather completes.
                for r in range(MPRE, NCORES):
                    nc.sync.dma_start(
                        kt_sb[:, :, r * NB:(r + 1) * NB],
                        agout[r, :, 0:DC * NB].rearrange(
                            "p (d n) -> p d n", d=DC))
                    nc.gpsimd.dma_start(
                        out=v_sb[:, 8 * r:8 * r + 8, :],
                        in_=agout[r, :, DC * NB:BCOLS].rearrange(
                            "p (c d) -> p c d", c=8))

                # ---- phase 2: attention in 4 superchunks; each final
                # chunk's PE epilogue (W1/W2 matmuls) is emitted AFTER the
                # next chunk's first attention units so the tensor queue
                # never stalls on the DVE evacuation/reciprocal chain ----
                pp = [0, 0]      # esum ping-pong state per i-half

                def attn_unit(t, ih, ctx_ps, first, last):
                    i0 = ih * IH
                    e_t = etile.tile([P, 2, IH], FP8, tag="et",
                                     name=f"e_{ih}_{t}")
                    for s in range(2):
                        jc = 2 * t + s
                        st_ps = ps_mm.tile([P, IH], F32, tag="st",
                                           name=f"st_{ih}_{t}_{s}")
                        for dp in range(DP):
                            nc.tensor.matmul(
                                st_ps[:],
                                kt_sb[:, 2 * dp:2 * dp + 2,
                                      jc * P:(jc + 1) * P],
                                qt_sb[:, 2 * dp:2 * dp + 2, i0:i0 + IH],
                                start=(dp == 0), stop=(dp == DP - 1),
                                perf_mode=DR)
                        nc.scalar.activation(
                            e_t[:, s], st_ps[:],
                            mybir.ActivationFunctionType.Exp,
                            bias=0.0, scale=SCALE)
                    src = pp[ih]
                    pp[ih] = 1 - src
                    nc.vector.tensor_tensor(esum[:, ih, pp[ih]], e_t[:],
                                            esum[:, ih, src],
                                            mybir.AluOpType.add)
                    for dc in range(DC):
                        nc.tensor.matmul(
                            ctx_ps[:, dc],
                            v_sb[:, 2 * t:2 * t + 2, dc * P:(dc + 1) * P],
                            e_t[:],
                            start=first, stop=last, perf_mode=DR)

                def epi_dve(ih, ctx_ps, den_ps):
                    # total sums + reciprocal; frees PSUM for the next chunk
                    ctxu = mlp.tile([P, DC, IH], BF16, tag="ctxu",
                                    name=f"ctxu_{ih}")
                    for dc in range(DC):
                        nc.vector.tensor_tensor(ctxu[:, dc], ctx_ps[:, dc],
                                                ctxacc[:, ih, dc],
                                                mybir.AluOpType.add)
                    recip_sb = mlp.tile([1, IH], F32, tag="recip",
                                        name=f"recip_{ih}")
                    nc.vector.reciprocal(recip_sb[:], den_ps[0:1])
                    r_sb = mlp.tile([P, IH], F32, tag="rsb",
                                    name=f"rsb_{ih}")
                    nc.gpsimd.partition_broadcast(r_sb[:], recip_sb[:])
                    return ctxu, r_sb

                def epi_pe(ih, ctxu, r_sb):
                    # g = W1^T ctxU;  h = relu(g/den + b1);  out = W2^T h
                    i0 = ih * IH
                    sc_ps = ps_mm.tile([1, IH], F32, tag="st",
                                       name=f"sc_{ih}")
                    h_sb = mlp.tile([P, MC, IH], BF16, tag="hsb",
                                    name=f"h_{ih}")
                    for mc in range(MC):
                        g_ps = ps_mm.tile([P, IH], F32, tag="st",
                                          name=f"g_{ih}_{mc}")
                        for dc in range(DC):
                            nc.tensor.matmul(g_ps[:], w1_sb[:, dc, mc],
                                             ctxu[:, dc],
                                             start=(dc == 0),
                                             stop=(dc == DC - 1))
                        gsc = mlp.tile([P, IH], BF16, tag="gsc",
                                       name=f"gsc_{ih}_{mc}")
                        nc.vector.tensor_tensor(gsc[:], g_ps[:], r_sb[:],
                                                mybir.AluOpType.mult)
                        nc.scalar.activation(
                            h_sb[:, mc], gsc[:],
                            mybir.ActivationFunctionType.Relu,
                            bias=b1_sb[:, mc:mc + 1], scale=1.0)
                    for mc in range(MC):
                        nc.tensor.matmul(sc_ps[:], w2_sb[:, mc:mc + 1],
                                         h_sb[:, mc],
                                         start=(mc == 0), stop=(mc == MC - 1))
                    nc.scalar.add(out_sb[:, i0:i0 + IH], sc_ps[:], b2_sb[:])

                pending = None
                for ih, blo, bhi in ((0, 0, MPRE), (1, 0, MPRE),
                                     (0, MPRE, NCORES), (1, MPRE, NCORES)):
                    ctx_ps = ps_ctx.tile([P, DC, IH], F32, tag="ctx",
                                         name=f"ctx_{ih}_{blo}")
                    t_first, t_last = 4 * blo, 4 * bhi - 1
                    for t in range(t_first, t_last + 1):
                        attn_unit(t, ih, ctx_ps, t == t_first, t == t_last)
                        if pending is not None and t == t_first + 1:
                            epi_pe(*pending)
                            pending = None
                    if bhi < NCORES:
                        # stash partial ctx sums; frees PSUM for next chunk
                        for dc in range(DC):
                            nc.vector.tensor_copy(ctxacc[:, ih, dc],
                                                  ctx_ps[:, dc])
                    else:
                        # 128-partition reduce of the E accumulator -> den
                        den_ps = ps_den.tile([1, IH], F32, tag="den",
                                             name=f"den_{ih}")
                        for s in range(2):
                            nc.tensor.matmul(den_ps[:], e8_w[:],
                                             esum[:, ih, pp[ih], s],
                                             start=(s == 0), stop=(s == 1))
                        pending = (ih,) + epi_dve(ih, ctx_ps, den_ps)
                epi_pe(*pending)

            nc.sync.dma_start(out[:], out_sb[:])

    nc.finalize()
    return nc


def _prep(inputs):
    """Host-side layout prep shared by all cores + per-core xq blocks."""
    f32 = np.float32
    bf16 = ml_dtypes.bfloat16
    fp8 = ml_dtypes.float8_e4m3
    x = np.ascontiguousarray(inputs["x"], dtype=f32)
    xt = np.ascontiguousarray(x.T)                                   # [IN, N]
    xt_r = np.ascontiguousarray(
        xt.reshape(KC, P, N).transpose(1, 0, 2).astype(fp8))         # [P, KC, N]

    def w_r(w):  # [IN, D] -> [P, KC, D], x8 scale into fp8 range
        return np.ascontiguousarray(
            (np.asarray(w, f32) * WS).reshape(KC, P, D)
            .transpose(1, 0, 2).astype(fp8))

    w1_r = np.ascontiguousarray(
        np.asarray(inputs["W1"], f32).reshape(DC, P, MC, P)
        .transpose(1, 0, 2, 3).astype(bf16))                         # [P, DC, MC, P]
    w2_r = np.ascontiguousarray(
        np.asarray(inputs["W2"], f32).reshape(MC, P).T.astype(bf16))  # [P, MC]

    def b_col(b, nchunks, scale=1.0):  # [nchunks*P] -> [P, nchunks]
        return np.ascontiguousarray(
            (np.asarray(b, f32) * scale).reshape(nchunks, P).T)

    shared = {
        "xp": np.ascontiguousarray(xt_r[:, :, 0:MPRE * NB]),
        "wq": w_r(inputs["Wq"]),
        "wk": w_r(inputs["Wk"]),
        "wv": w_r(inputs["Wv"]),
        "w1": w1_r,
        "w2": w2_r,
        "bq": b_col(inputs["bq"], DC, WS),
        "bk": b_col(inputs["bk"], DC, WS),
        "bv": np.ascontiguousarray(
            np.broadcast_to(np.asarray(inputs["bv"], f32) * WS, (P, D))),
        "b1": b_col(inputs["b1"], MC),
        "b2": np.asarray(inputs["b2"], f32).reshape(1, 1),
    }
    xqs = [np.ascontiguousarray(xt_r[:, :, c * NB:(c + 1) * NB])
           for c in range(NCORES)]
    return shared, xqs


def kernel(**inputs) -> np.ndarray:
    if "nc" not in _CACHE:
        _CACHE["nc"] = _build()
    nc = _CACHE["nc"]
    shared, xqs = _prep(inputs)
    in_maps = [dict(shared, xq=xqs[c]) for c in range(NCORES)]
    res = run_bass_kernel_spmd(nc, in_maps, core_ids=list(range(NCORES)))
    return np.concatenate([res.results[c]["out"][0] for c in range(NCORES)])
